# revision 37
# baseline (speedup 1.0000x reference)
"""Trainium2 Bass kernel for LlamaAttention (B=1, S=2048, H=4096, 32 heads).

Sharding: tensor-parallel over heads. 8 cores x 4 heads. Each core:
  - QKV projections in bf16 (1 cyc/out-col on the PE at 2.4 GHz; fp8
    DoubleRow measured at the same out-col rate on HW, so bf16 wins
    once accuracy needs >1 fp8 pass). Wq resident in SBUF as a single
    p-major tile; Wk/Wv streamed per chunk in g-split tiles; hidden
    states streamed per chunk with next-chunk prefetch emitted before
    the attention phase. Preloads split across the sync/scalar/gpsimd
    dispatch queues (packets of one DMA spread over all 16 engines).
  - head-dim PAIR accumulation in [128,2,CH] PSUM tiles (2 banks) with
    early eviction so Q/K/V pass transitions don't stall; V pass pair 1
    trails pair 0 by 4 k-steps; RoPE on Q^T/K^T (rotate-half =
    partition swap via SBUF DMA, dispatched on gpsimd).
  - causal attention in transposed layout (keys on partitions), bf16
    scores / exp / PV; j-tile pairs share one [128,1024] exp emitted
    after both score matmuls (avoids a tile-granular WAR stall);
    globally software-pipelined across heads; fully-masked blocks
    skipped and diagonal tiles narrowed to queries >= tile start;
    softmax without max subtraction; column sums via ones-matmul;
    1/sum via fast-reciprocal + gpsimd partition_broadcast.
  - attention output kept in SBUF (bf16, per-chunk tiles); o_proj bf16
    with WO loaded into WQ's dead SBUF region during the last
    attention chunk; partial po written bf16.
  Host sums the 8 partials and transposes. No collectives.
"""

import os
import sys

if "/opt/trn_rl_repo" not in sys.path:
    sys.path.insert(0, "/opt/trn_rl_repo")

import numpy as np
import ml_dtypes

from concourse import bacc, mybir, tile
from concourse import bass
from concourse.bass_utils import run_bass_kernel_spmd

F32 = mybir.dt.float32
F32R = mybir.dt.float32r
BF16 = mybir.dt.bfloat16
EXPF = mybir.ActivationFunctionType.Exp

N_CORES = 8
S = 2048
H = 4096
N_HEADS = 32
D = 128                      # head dim
HPC = N_HEADS // N_CORES     # heads per core = 4
HC = HPC * D                 # per-core hidden slice = 512
CH = 512                     # seq chunk width
NCH = S // CH                # 4 chunks
KT_TILES = H // 128          # 32 contraction tiles for projections
SJT = S // 128               # 16 seq j-tiles
ROPE_BASE = 10000.0
NEG = -1.0e9

BFNP = ml_dtypes.bfloat16

last_exec_time_ns = None


def _r(x):
    return np.ascontiguousarray(x, dtype=np.float32)


def _b(x):
    return np.ascontiguousarray(np.asarray(x, np.float32).astype(BFNP))


def _pmajor(a, kt):
    """[kt*128, C] -> [128, kt, C] partition-major layout."""
    R, C = a.shape
    return np.ascontiguousarray(
        np.asarray(a).reshape(kt, 128, C).transpose(1, 0, 2))


def _build(causal: bool):
    nc = bacc.Bacc("TRN2", target_bir_lowering=False, debug=False,
                   num_devices=N_CORES)
    htb = nc.dram_tensor("htb", [128, KT_TILES, S], BF16,
                         kind="ExternalInput")
    wqb = nc.dram_tensor("wqb", [128, KT_TILES, HC], BF16,
                         kind="ExternalInput")
    wkb = nc.dram_tensor("wkb", [128, KT_TILES, HC], BF16,
                         kind="ExternalInput")
    wvb = nc.dram_tensor("wvb", [KT_TILES // 4, 128, 4 * HC], BF16,
                         kind="ExternalInput")
    wob = nc.dram_tensor("wob", [128, HPC, H], BF16, kind="ExternalInput")
    cosb = nc.dram_tensor("cosb", [D, S], BF16, kind="ExternalInput")
    sinb = nc.dram_tensor("sinb", [D, S], BF16, kind="ExternalInput")
    if causal:
        mband = nc.dram_tensor("mband", [128, 896], BF16,
                               kind="ExternalInput")
    else:
        maskT = nc.dram_tensor("maskT", [S, S], F32, kind="ExternalInput")
    po = nc.dram_tensor("po", [H, S], BF16, kind="ExternalOutput")

    def mm(out, lhsT, rhs, start, stop):
        nc.tensor.matmul(out, lhsT, rhs, start=start, stop=stop)

    from contextlib import ExitStack
    with tile.TileContext(nc) as tc:
        at_pool_cm = tc.tile_pool(name="at", bufs=NCH)
        at_pool = at_pool_cm.__enter__()
        AT = [at_pool.tile([128, HPC, CH], BF16, tag="at", name=f"AT{i}")
              for i in range(NCH)]

        es_res = ExitStack()
        kt_pool = es_res.enter_context(tc.tile_pool(name="kt", bufs=HPC))
        v_pool = es_res.enter_context(tc.tile_pool(name="v", bufs=SJT))
        wqk_pool = es_res.enter_context(tc.tile_pool(name="wqk", bufs=1))
        KT = [kt_pool.tile([128, S], BF16, tag="kt", name=f"KT{i}")
              for i in range(HPC)]
        V = [v_pool.tile([128, HC], BF16, tag="v", name=f"V{i}")
             for i in range(SJT)]
        WQ = wqk_pool.tile([128, KT_TILES, HC], BF16, tag="wq", name="WQ")

        with tc.tile_pool(name="qtc", bufs=4) as qtp, \
             tc.tile_pool(name="ht", bufs=10) as htp, \
             tc.tile_pool(name="wvs", bufs=3) as wvp, \
             tc.tile_pool(name="wks", bufs=3) as wkp, \
             tc.tile_pool(name="rope", bufs=2) as rp, \
             tc.tile_pool(name="aconst", bufs=1) as cpool, \
             tc.tile_pool(name="aes", bufs=2) as esp, \
             tc.tile_pool(name="am", bufs=1 if causal else 4) as mpool, \
             tc.tile_pool(name="ar", bufs=2) as rpool, \
             tc.tile_pool(name="mainps", bufs=2, space="PSUM") as psp:
            def load_ht(c, fine=False):
                gs = []
                for g in range(8):
                    t_ = htp.tile([128, 4, CH], BF16, tag="ht", name="ht_g")
                    if fine:
                        for u in range(2):
                            nc.sync.dma_start(
                                out=t_[:, bass.ts(u, 2), :],
                                in_=htb[:, 4 * g + 2 * u:4 * g + 2 * u + 2,
                                        bass.ts(c, CH)])
                    else:
                        nc.sync.dma_start(out=t_[:],
                                          in_=htb[:, bass.ts(g, 4),
                                                  bass.ts(c, CH)])
                    gs.append(t_)
                return gs

            def load_wk(c):
                gs = []
                for g in range(8):
                    t_ = wkp.tile([128, 4, HC], BF16, tag="wk", name="wk_g")
                    nc.scalar.dma_start(out=t_[:],
                                        in_=wkb[:, bass.ts(g, 4), :])
                    gs.append(t_)
                return gs

            def load_wv(c):
                gs = []
                for g in range(8):
                    t_ = wvp.tile([128, 4, HC], BF16, tag="wv", name="wv_g")
                    nc.gpsimd.dma_start(out=t_[:], in_=wvb[g])
                    gs.append(t_)
                return gs

            # preload: ht chunk 0 on sync, WQ on scalar — the queues run
            # transfers concurrently and trickle-feed the first Q pass
            HT = load_ht(0, fine=True)
            for g in range(16):
                nc.scalar.dma_start(out=WQ[:, bass.ts(g, 2), :],
                                    in_=wqb[:, bass.ts(g, 2), :])
            ones_col32 = cpool.tile([128, 1], F32, tag="oc32")
            nc.vector.memset(ones_col32[:], 1.0)
            ones_col = cpool.tile([128, 1], BF16, tag="oc")
            nc.vector.tensor_copy(ones_col[:], ones_col32[:])
            cosT = cpool.tile([128, S], BF16, tag="cos", name="cosT")
            sinT = cpool.tile([128, S], BF16, tag="sin", name="sinT")
            nc.scalar.dma_start(out=cosT[:], in_=cosb[:, :])
            nc.scalar.dma_start(out=sinT[:], in_=sinb[:, :])
            if causal:
                mb = cpool.tile([128, 896], BF16, tag="mb", name="mb")
                nc.scalar.dma_start(out=mb[:], in_=mband[:, :])

            def rope_evict(ps, dst_ap, c):
                # dst = psum*cos + shift(psum)*sin_signed
                cosc = cosT[:, bass.ts(c, CH)]
                sinc = sinT[:, bass.ts(c, CH)]
                raw = rp.tile([128, CH], BF16, tag="raw", name="raw")
                nc.scalar.copy(out=raw[:], in_=ps)
                shf = rp.tile([128, CH], BF16, tag="shf", name="shf")
                nc.gpsimd.dma_start(out=shf[0:64, :], in_=raw[64:128, :])
                nc.gpsimd.dma_start(out=shf[64:128, :], in_=raw[0:64, :])
                tmp = rp.tile([128, CH], BF16, tag="tmp", name="tmp")
                nc.vector.tensor_mul(tmp[:], shf[:], sinc)
                nc.vector.tensor_mul(dst_ap, raw[:], cosc)
                nc.vector.tensor_add(dst_ap, dst_ap, tmp[:])

            for c in range(NCH):
                WKg = load_wk(c)
                WVg = load_wv(c)
                # ---- Q pass (head-dim pairs, early evict) ----
                QTc = [qtp.tile([128, CH], BF16, tag="qtc", name=f"QTc{i}")
                       for i in range(HPC)]
                for dp in range(2):
                    qp2 = psp.tile([128, 2, CH], F32, tag="big",
                                   name="qp2")
                    for k in range(KT_TILES):
                        st, sp = (k == 0), (k == KT_TILES - 1)
                        for t in range(2):
                            d = 2 * dp + t
                            mm(qp2[:, t, :], WQ[:, k, bass.ts(d, 128)],
                               HT[k // 4][:, k % 4, :], st, sp)
                    for t in range(2):
                        rope_evict(qp2[:, t, :], QTc[2 * dp + t][:], c)
                # ---- K pass ----
                for dp in range(2):
                    kp2 = psp.tile([128, 2, CH], F32, tag="big",
                                   name="kp2")
                    for k in range(KT_TILES):
                        st, sp = (k == 0), (k == KT_TILES - 1)
                        for t in range(2):
                            d = 2 * dp + t
                            mm(kp2[:, t, :],
                               WKg[k // 4][:, k % 4, bass.ts(d, 128)],
                               HT[k // 4][:, k % 4, :], st, sp)
                    for t in range(2):
                        rope_evict(kp2[:, t, :],
                                   KT[2 * dp + t][:, bass.ts(c, CH)], c)
                # ---- V pass (wv streamed; pair 1 trails pair 0 by
                # 4 k-steps so PSUM evictions hide under matmuls) ----
                vp2 = [None, None]
                vp2[0] = psp.tile([128, 2, CH], F32, tag="big",
                                  name="vp2_0")
                SKEW = 4
                for kk in range(KT_TILES + SKEW):
                    if kk < KT_TILES:
                        k = kk
                        if kk == SKEW:
                            vp2[1] = psp.tile([128, 2, CH], F32,
                                              tag="big", name="vp2_1")
                        for jl in range(2):
                            mm(vp2[0][:, jl, :],
                               HT[k // 4][:, k % 4, bass.ts(jl, 128)],
                               WVg[k // 4][:, k % 4, :],
                               k == 0, k == KT_TILES - 1)
                        if kk == KT_TILES - 1:
                            for jl in range(2):
                                nc.scalar.copy(
                                    out=V[4 * c + jl][:],
                                    in_=vp2[0][:, jl, :])
                    if kk >= SKEW:
                        k = kk - SKEW
                        for jl in range(2, 4):
                            mm(vp2[1][:, jl - 2, :],
                               HT[k // 4][:, k % 4, bass.ts(jl, 128)],
                               WVg[k // 4][:, k % 4, :],
                               k == 0, k == KT_TILES - 1)
                for jl in range(2, 4):
                    nc.scalar.copy(out=V[4 * c + jl][:],
                                   in_=vp2[1][:, jl - 2, :])

                # prefetch next chunk's hidden tiles during attention;
                # last chunk: overwrite dead WQ SBUF with WO instead
                # (same byte layout: WO[:, kl, n*128+x] lives at
                # WQ[:, kl*8 + n//4, (n%4)*128 + x])
                if c + 1 < NCH:
                    HT = load_ht(c + 1)
                else:
                    for kl in range(HPC):
                        nc.sync.dma_start(
                            out=WQ[:, bass.ts(kl, 8), :],
                            in_=wob[:, kl, :])

                # ---- attention for i-chunk c (K/V chunks <= c) ----
                ic = c
                jp_max = (2 * ic + 2) if causal else (SJT // 2)

                def finish_head(h_, sum_, o_):
                    rsum = rpool.tile([1, CH], F32, tag="rs", name="rsum")
                    rscr = rpool.tile([1, CH], F32, tag="rscr",
                                      name="rscr")
                    nc.vector.reciprocal_approx_accurate(
                        out=rsum[:], in_=sum_[:], scratch=rscr[:])
                    rb = rpool.tile([128, CH], F32, tag="rb", name="rb")
                    nc.gpsimd.partition_broadcast(rb[:], rsum[:])
                    nc.vector.tensor_mul(
                        AT[ic][:, h_, :], o_[:], rb[:])

                pend = []

                def drain_one():
                    h_, jp_, q0s_, es2_, sum_, o_ = pend.pop(0)
                    # both sum matmuls adjacent: one ones-LDWEIGHTS per pair
                    last = False
                    for t in range(2):
                        j = 2 * jp_ + t
                        q0 = q0s_[t]
                        last = (j == 2 * jp_max - 1)
                        mm(sum_[:, q0:CH], ones_col[:],
                           es2_[:, t, q0:CH], j == 0, last)
                    for t in range(2):
                        j = 2 * jp_ + t
                        q0 = q0s_[t]
                        mm(o_[:, q0:CH], V[j][:, bass.ts(h_, 128)],
                           es2_[:, t, q0:CH], j == 0,
                           j == 2 * jp_max - 1)
                    if last:
                        finish_head(h_, sum_, o_)

                for h in range(HPC):
                    sum_ps = psp.tile([1, CH], F32, tag="sum", bufs=2,
                                      name="sum_ps")
                    o_ps = psp.tile([128, CH], F32, tag="o", bufs=2,
                                    name="o_ps")
                    for jp in range(jp_max):
                        s2 = psp.tile([128, 2, CH], F32, tag="big",
                                      name="s2")
                        # scores first (both halves), mask adds after —
                        # avoids a tile-granularity WAR stall on s2
                        q0s = []
                        for t in range(2):
                            j = 2 * jp + t
                            tl = j - 4 * ic
                            q0 = tl * 128 if (causal and tl > 0) else 0
                            q0s.append(q0)
                            mm(s2[:, t, q0:CH],
                               KT[h][:, bass.ts(j, 128)],
                               QTc[h][:, q0:CH], True, True)
                        for t in range(2):
                            j = 2 * jp + t
                            if causal:
                                if j >= 4 * ic:
                                    q0 = q0s[t]
                                    nc.vector.tensor_add(
                                        s2[:, t, q0:CH], s2[:, t, q0:CH],
                                        mb[:, 384:384 + CH - q0])
                            else:
                                mt = mpool.tile([128, CH], F32, tag="mt",
                                                name="mt")
                                nc.sync.dma_start(
                                    out=mt[:],
                                    in_=maskT[bass.ts(j, 128),
                                              bass.ts(ic, CH)])
                                nc.vector.tensor_add(s2[:, t, :],
                                                     s2[:, t, :], mt[:])
                        es2 = esp.tile([128, 2, CH], BF16, tag="es",
                                       name="es2")
                        nc.scalar.activation(es2[:], s2[:], EXPF)
                        pend.append((h, jp, q0s, es2, sum_ps, o_ps))
                        if len(pend) > 1:
                            drain_one()
                while pend:
                    drain_one()
        # ---------- o_proj  po = wo^T @ attnT (WO aliased in WQ) ----------
        with tc.tile_pool(name="o_out", bufs=8) as outp, \
             tc.tile_pool(name="o_ps", bufs=8, space="PSUM") as psp:
            NB = 4           # n-tiles per block; kl-outer within a block
            for ic in range(NCH):
                for nb in range(H // 128 // NB):
                    pps = [psp.tile([128, CH], F32, tag="ps", name="pps")
                           for _ in range(NB)]
                    for kl in range(HPC):
                        for i in range(NB):
                            n = nb * NB + i
                            wo_ap = WQ[:, kl * 8 + n // 4,
                                       (n % 4) * 128:(n % 4) * 128 + 128]
                            mm(pps[i][:], wo_ap,
                               AT[ic][:, kl, :],
                               kl == 0, kl == HPC - 1)
                    for i in range(NB):
                        n = nb * NB + i
                        ot = outp.tile([128, CH], BF16, tag="ot", name="ot")
                        nc.scalar.copy(out=ot[:], in_=pps[i][:])
                        nc.gpsimd.dma_start(
                            out=po[bass.ts(n, 128), bass.ts(ic, CH)],
                            in_=ot[:])
        es_res.close()
        at_pool_cm.__exit__(None, None, None)
    nc.compile()
    return nc


_CACHE = {}


def _get_nc(causal):
    if causal not in _CACHE:
        _CACHE[causal] = _build(causal)
    return _CACHE[causal]


def kernel(hidden_states, attention_mask, position_ids, Wq, Wk, Wv, Wo):
    global last_exec_time_ns
    B, S_, H_ = hidden_states.shape
    assert (B, S_, H_) == (1, S, H)
    hs = np.asarray(hidden_states, dtype=np.float32)
    mask = np.asarray(attention_mask, dtype=np.float32)[0, 0]
    pos = np.asarray(position_ids)[0].astype(np.float64)

    # causal-mask fast path check
    iu = np.triu_indices(S, k=1)
    il = np.tril_indices(S, k=0)
    causal = bool(np.all(mask[il] == 0.0) and np.all(mask[iu] <= -1e30))

    hT = np.asarray(hs[0]).T               # [H, S]
    scale = 1.0 / np.sqrt(D)

    inv_freq = 1.0 / (ROPE_BASE ** (np.arange(0, D, 2, dtype=np.float64) / D))
    ang = pos[None, :] * np.concatenate([inv_freq, inv_freq])[:, None]  # [D,S]
    cosb = _b(np.cos(ang))
    sgn = np.ones((D, 1)); sgn[: D // 2] = -1.0
    sinb = _b(np.sin(ang) * sgn)

    htb = _pmajor(_b(hT), KT_TILES)
    wq_s = _b(np.asarray(Wq, np.float64) * scale)
    wk_b = _b(Wk)
    wv_b = _b(Wv)
    wo_b = _b(Wo)

    if causal:
        # band mask tile [128, 896]: mb[r, y] = NEG iff r > y - 384
        rr = np.arange(128)[:, None]
        yy = np.arange(896)[None, :]
        mband = np.ascontiguousarray(
            np.where(rr > yy - 384, NEG, 0.0).astype(BFNP))
    else:
        maskT = _r(mask.T)

    nc = _get_nc(causal)
    in_maps = []
    for c in range(N_CORES):
        sl = slice(c * HC, (c + 1) * HC)
        m = {
            "htb": htb,
            "wqb": _pmajor(wq_s[:, sl], KT_TILES),
            "wkb": _pmajor(wk_b[:, sl], KT_TILES),
            "wvb": np.ascontiguousarray(
                wv_b[:, sl].reshape(8, 4, 128, HC).transpose(0, 2, 1, 3)
                .reshape(8, 128, 4 * HC)),
            "wob": _pmajor(wo_b[sl, :], HPC),
            "cosb": cosb,
            "sinb": sinb,
        }
        if causal:
            m["mband"] = mband
        else:
            m["maskT"] = maskT
        in_maps.append(m)

    trace = bool(int(os.environ.get("BASS_KERNEL_TRACE", "0")))
    kw = {}
    if trace:
        kw["trace"] = True
        kw["tmpdir"] = os.environ.get("BASS_KERNEL_TRACE_DIR") or None
    res = run_bass_kernel_spmd(nc, in_maps, list(range(N_CORES)), **kw)
    last_exec_time_ns = res.exec_time_ns

    acc = np.zeros((H, S), dtype=np.float32)
    for c in range(N_CORES):
        acc += res.results[c]["po"].astype(np.float32)
    out = acc.T.reshape(1, S, H)
    return out


# revision 38
# speedup vs baseline: 1.0213x; 1.0213x over previous
"""Trainium2 Bass kernel for LlamaAttention (B=1, S=2048, H=4096, 32 heads).

Sharding: tensor-parallel over heads. 8 cores x 4 heads. Each core:
  - QKV projections in bf16 (1 cyc/out-col on the PE at 2.4 GHz; fp8
    DoubleRow measured at the same out-col rate on HW, so bf16 wins
    once accuracy needs >1 fp8 pass). Wq resident in SBUF as a single
    p-major tile; Wk/Wv streamed per chunk in g-split tiles; hidden
    states streamed per chunk with next-chunk prefetch emitted before
    the attention phase. Preloads split across the sync/scalar/gpsimd
    dispatch queues (packets of one DMA spread over all 16 engines).
  - head-dim PAIR accumulation in [128,2,CH] PSUM tiles (2 banks) with
    early eviction so Q/K/V pass transitions don't stall; V pass pair 1
    trails pair 0 by 4 k-steps; RoPE on Q^T/K^T (rotate-half =
    partition swap via SBUF DMA, dispatched on gpsimd).
  - causal attention in transposed layout (keys on partitions), bf16
    scores / exp / PV; j-tile pairs share one [128,1024] exp emitted
    after both score matmuls (avoids a tile-granular WAR stall);
    globally software-pipelined across heads; fully-masked blocks
    skipped and diagonal tiles narrowed to queries >= tile start;
    softmax without max subtraction; column sums via ones-matmul;
    1/sum via fast-reciprocal + gpsimd partition_broadcast.
  - attention output kept in SBUF (bf16, per-chunk tiles); o_proj bf16
    with WO loaded into WQ's dead SBUF region during the last
    attention chunk; partial po written bf16.
  Host sums the 8 partials and transposes. No collectives.
"""

import os
import sys

if "/opt/trn_rl_repo" not in sys.path:
    sys.path.insert(0, "/opt/trn_rl_repo")

import numpy as np
import ml_dtypes

from concourse import bacc, mybir, tile
from concourse import bass
from concourse.bass_utils import run_bass_kernel_spmd

F32 = mybir.dt.float32
F32R = mybir.dt.float32r
BF16 = mybir.dt.bfloat16
EXPF = mybir.ActivationFunctionType.Exp

N_CORES = 8
S = 2048
H = 4096
N_HEADS = 32
D = 128                      # head dim
HPC = N_HEADS // N_CORES     # heads per core = 4
HC = HPC * D                 # per-core hidden slice = 512
CH = 512                     # seq chunk width
NCH = S // CH                # 4 chunks
KT_TILES = H // 128          # 32 contraction tiles for projections
SJT = S // 128               # 16 seq j-tiles
ROPE_BASE = 10000.0
NEG = -1.0e9

BFNP = ml_dtypes.bfloat16

last_exec_time_ns = None


def _r(x):
    return np.ascontiguousarray(x, dtype=np.float32)


def _b(x):
    return np.ascontiguousarray(np.asarray(x, np.float32).astype(BFNP))


def _pmajor(a, kt):
    """[kt*128, C] -> [128, kt, C] partition-major layout."""
    R, C = a.shape
    return np.ascontiguousarray(
        np.asarray(a).reshape(kt, 128, C).transpose(1, 0, 2))


def _build(causal: bool):
    nc = bacc.Bacc("TRN2", target_bir_lowering=False, debug=False,
                   num_devices=N_CORES)
    htb = nc.dram_tensor("htb", [128, KT_TILES, S], BF16,
                         kind="ExternalInput")
    wqb = nc.dram_tensor("wqb", [128, KT_TILES, HC], BF16,
                         kind="ExternalInput")
    wkb = nc.dram_tensor("wkb", [128, KT_TILES, HC], BF16,
                         kind="ExternalInput")
    wvb = nc.dram_tensor("wvb", [KT_TILES // 4, 128, 4 * HC], BF16,
                         kind="ExternalInput")
    wob = nc.dram_tensor("wob", [128, HPC, H], BF16, kind="ExternalInput")
    cosb = nc.dram_tensor("cosb", [D, S], BF16, kind="ExternalInput")
    sinb = nc.dram_tensor("sinb", [D, S], BF16, kind="ExternalInput")
    if causal:
        mband = nc.dram_tensor("mband", [128, 896], BF16,
                               kind="ExternalInput")
    else:
        maskT = nc.dram_tensor("maskT", [S, S], F32, kind="ExternalInput")
    po = nc.dram_tensor("po", [H, S], BF16, kind="ExternalOutput")

    def mm(out, lhsT, rhs, start, stop):
        nc.tensor.matmul(out, lhsT, rhs, start=start, stop=stop)

    from contextlib import ExitStack
    with tile.TileContext(nc) as tc:
        at_pool_cm = tc.tile_pool(name="at", bufs=NCH)
        at_pool = at_pool_cm.__enter__()
        AT = [at_pool.tile([128, HPC, CH], BF16, tag="at", name=f"AT{i}")
              for i in range(NCH)]

        es_res = ExitStack()
        kt_pool = es_res.enter_context(tc.tile_pool(name="kt", bufs=HPC))
        v_pool = es_res.enter_context(tc.tile_pool(name="v", bufs=SJT))
        wqk_pool = es_res.enter_context(tc.tile_pool(name="wqk", bufs=1))
        KT = [kt_pool.tile([128, S], BF16, tag="kt", name=f"KT{i}")
              for i in range(HPC)]
        V = [v_pool.tile([128, HC], BF16, tag="v", name=f"V{i}")
             for i in range(SJT)]
        WQ = wqk_pool.tile([128, KT_TILES, HC], BF16, tag="wq", name="WQ")

        with tc.tile_pool(name="qtc", bufs=4) as qtp, \
             tc.tile_pool(name="ht", bufs=10) as htp, \
             tc.tile_pool(name="wvs", bufs=3) as wvp, \
             tc.tile_pool(name="wks", bufs=3) as wkp, \
             tc.tile_pool(name="rope", bufs=2) as rp, \
             tc.tile_pool(name="aconst", bufs=1) as cpool, \
             tc.tile_pool(name="aes", bufs=2) as esp, \
             tc.tile_pool(name="am", bufs=1 if causal else 4) as mpool, \
             tc.tile_pool(name="ar", bufs=2) as rpool, \
             tc.tile_pool(name="mainps", bufs=2, space="PSUM") as psp:
            def load_ht(c, fine=False):
                gs = []
                for g in range(8):
                    t_ = htp.tile([128, 4, CH], BF16, tag="ht", name="ht_g")
                    if fine:
                        for u in range(2):
                            nc.sync.dma_start(
                                out=t_[:, bass.ts(u, 2), :],
                                in_=htb[:, 4 * g + 2 * u:4 * g + 2 * u + 2,
                                        bass.ts(c, CH)])
                    else:
                        nc.sync.dma_start(out=t_[:],
                                          in_=htb[:, bass.ts(g, 4),
                                                  bass.ts(c, CH)])
                    gs.append(t_)
                return gs

            def load_wk(c):
                # chunk 0: scalar queue is backed up with the WQ preload,
                # so dispatch from gpsimd (wv isn't needed until ~70us)
                eng = nc.gpsimd if c == 0 else nc.scalar
                gs = []
                for g in range(8):
                    t_ = wkp.tile([128, 4, HC], BF16, tag="wk", name="wk_g")
                    eng.dma_start(out=t_[:],
                                  in_=wkb[:, bass.ts(g, 4), :])
                    gs.append(t_)
                return gs

            def load_wv(c):
                gs = []
                for g in range(8):
                    t_ = wvp.tile([128, 4, HC], BF16, tag="wv", name="wv_g")
                    nc.gpsimd.dma_start(out=t_[:], in_=wvb[g])
                    gs.append(t_)
                return gs

            # preload: ht chunk 0 on sync, WQ on scalar — the queues run
            # transfers concurrently and trickle-feed the first Q pass
            HT = load_ht(0, fine=True)
            for g in range(16):
                nc.scalar.dma_start(out=WQ[:, bass.ts(g, 2), :],
                                    in_=wqb[:, bass.ts(g, 2), :])
            ones_col32 = cpool.tile([128, 1], F32, tag="oc32")
            nc.vector.memset(ones_col32[:], 1.0)
            ones_col = cpool.tile([128, 1], BF16, tag="oc")
            nc.vector.tensor_copy(ones_col[:], ones_col32[:])
            cosT = cpool.tile([128, S], BF16, tag="cos", name="cosT")
            sinT = cpool.tile([128, S], BF16, tag="sin", name="sinT")
            nc.scalar.dma_start(out=cosT[:], in_=cosb[:, :])
            nc.scalar.dma_start(out=sinT[:], in_=sinb[:, :])
            if causal:
                mb = cpool.tile([128, 896], BF16, tag="mb", name="mb")
                nc.scalar.dma_start(out=mb[:], in_=mband[:, :])

            def rope_evict(ps, dst_ap, c):
                # dst = psum*cos + shift(psum)*sin_signed
                cosc = cosT[:, bass.ts(c, CH)]
                sinc = sinT[:, bass.ts(c, CH)]
                raw = rp.tile([128, CH], BF16, tag="raw", name="raw")
                nc.scalar.copy(out=raw[:], in_=ps)
                shf = rp.tile([128, CH], BF16, tag="shf", name="shf")
                nc.gpsimd.dma_start(out=shf[0:64, :], in_=raw[64:128, :])
                nc.gpsimd.dma_start(out=shf[64:128, :], in_=raw[0:64, :])
                tmp = rp.tile([128, CH], BF16, tag="tmp", name="tmp")
                nc.vector.tensor_mul(tmp[:], shf[:], sinc)
                nc.vector.tensor_mul(dst_ap, raw[:], cosc)
                nc.vector.tensor_add(dst_ap, dst_ap, tmp[:])

            for c in range(NCH):
                WKg = load_wk(c)
                WVg = load_wv(c)
                # ---- Q pass (head-dim pairs, early evict) ----
                QTc = [qtp.tile([128, CH], BF16, tag="qtc", name=f"QTc{i}")
                       for i in range(HPC)]
                for dp in range(2):
                    qp2 = psp.tile([128, 2, CH], F32, tag="big",
                                   name="qp2")
                    for k in range(KT_TILES):
                        st, sp = (k == 0), (k == KT_TILES - 1)
                        for t in range(2):
                            d = 2 * dp + t
                            mm(qp2[:, t, :], WQ[:, k, bass.ts(d, 128)],
                               HT[k // 4][:, k % 4, :], st, sp)
                    for t in range(2):
                        rope_evict(qp2[:, t, :], QTc[2 * dp + t][:], c)
                # ---- K pass ----
                for dp in range(2):
                    kp2 = psp.tile([128, 2, CH], F32, tag="big",
                                   name="kp2")
                    for k in range(KT_TILES):
                        st, sp = (k == 0), (k == KT_TILES - 1)
                        for t in range(2):
                            d = 2 * dp + t
                            mm(kp2[:, t, :],
                               WKg[k // 4][:, k % 4, bass.ts(d, 128)],
                               HT[k // 4][:, k % 4, :], st, sp)
                    for t in range(2):
                        rope_evict(kp2[:, t, :],
                                   KT[2 * dp + t][:, bass.ts(c, CH)], c)
                # ---- V pass (wv streamed; pair 1 trails pair 0 by
                # 4 k-steps so PSUM evictions hide under matmuls) ----
                vp2 = [None, None]
                vp2[0] = psp.tile([128, 2, CH], F32, tag="big",
                                  name="vp2_0")
                SKEW = 4
                for kk in range(KT_TILES + SKEW):
                    if kk < KT_TILES:
                        k = kk
                        if kk == SKEW:
                            vp2[1] = psp.tile([128, 2, CH], F32,
                                              tag="big", name="vp2_1")
                        for jl in range(2):
                            mm(vp2[0][:, jl, :],
                               HT[k // 4][:, k % 4, bass.ts(jl, 128)],
                               WVg[k // 4][:, k % 4, :],
                               k == 0, k == KT_TILES - 1)
                        if kk == KT_TILES - 1:
                            for jl in range(2):
                                nc.scalar.copy(
                                    out=V[4 * c + jl][:],
                                    in_=vp2[0][:, jl, :])
                    if kk >= SKEW:
                        k = kk - SKEW
                        for jl in range(2, 4):
                            mm(vp2[1][:, jl - 2, :],
                               HT[k // 4][:, k % 4, bass.ts(jl, 128)],
                               WVg[k // 4][:, k % 4, :],
                               k == 0, k == KT_TILES - 1)
                for jl in range(2, 4):
                    nc.scalar.copy(out=V[4 * c + jl][:],
                                   in_=vp2[1][:, jl - 2, :])

                # prefetch next chunk's hidden tiles during attention;
                # last chunk: overwrite dead WQ SBUF with WO instead
                # (same byte layout: WO[:, kl, n*128+x] lives at
                # WQ[:, kl*8 + n//4, (n%4)*128 + x])
                if c + 1 < NCH:
                    HT = load_ht(c + 1)
                else:
                    for kl in range(HPC):
                        nc.sync.dma_start(
                            out=WQ[:, bass.ts(kl, 8), :],
                            in_=wob[:, kl, :])

                # ---- attention for i-chunk c (K/V chunks <= c) ----
                ic = c
                jp_max = (2 * ic + 2) if causal else (SJT // 2)

                def finish_head(h_, sum_, o_):
                    rsum = rpool.tile([1, CH], F32, tag="rs", name="rsum")
                    rscr = rpool.tile([1, CH], F32, tag="rscr",
                                      name="rscr")
                    nc.vector.reciprocal_approx_accurate(
                        out=rsum[:], in_=sum_[:], scratch=rscr[:])
                    rb = rpool.tile([128, CH], F32, tag="rb", name="rb")
                    nc.gpsimd.partition_broadcast(rb[:], rsum[:])
                    nc.vector.tensor_mul(
                        AT[ic][:, h_, :], o_[:], rb[:])

                pend = []

                def drain_one():
                    h_, jp_, q0s_, es2_, sum_, o_ = pend.pop(0)
                    # both sum matmuls adjacent: one ones-LDWEIGHTS per pair
                    last = False
                    for t in range(2):
                        j = 2 * jp_ + t
                        q0 = q0s_[t]
                        last = (j == 2 * jp_max - 1)
                        mm(sum_[:, q0:CH], ones_col[:],
                           es2_[:, t, q0:CH], j == 0, last)
                    for t in range(2):
                        j = 2 * jp_ + t
                        q0 = q0s_[t]
                        mm(o_[:, q0:CH], V[j][:, bass.ts(h_, 128)],
                           es2_[:, t, q0:CH], j == 0,
                           j == 2 * jp_max - 1)
                    if last:
                        finish_head(h_, sum_, o_)

                for h in range(HPC):
                    sum_ps = psp.tile([1, CH], F32, tag="sum", bufs=2,
                                      name="sum_ps")
                    o_ps = psp.tile([128, CH], F32, tag="o", bufs=2,
                                    name="o_ps")
                    for jp in range(jp_max):
                        s2 = psp.tile([128, 2, CH], F32, tag="big",
                                      name="s2")
                        # scores first (both halves), mask adds after —
                        # avoids a tile-granularity WAR stall on s2
                        q0s = []
                        for t in range(2):
                            j = 2 * jp + t
                            tl = j - 4 * ic
                            q0 = tl * 128 if (causal and tl > 0) else 0
                            q0s.append(q0)
                            mm(s2[:, t, q0:CH],
                               KT[h][:, bass.ts(j, 128)],
                               QTc[h][:, q0:CH], True, True)
                        for t in range(2):
                            j = 2 * jp + t
                            if causal:
                                if j >= 4 * ic:
                                    q0 = q0s[t]
                                    nc.vector.tensor_add(
                                        s2[:, t, q0:CH], s2[:, t, q0:CH],
                                        mb[:, 384:384 + CH - q0])
                            else:
                                mt = mpool.tile([128, CH], F32, tag="mt",
                                                name="mt")
                                nc.sync.dma_start(
                                    out=mt[:],
                                    in_=maskT[bass.ts(j, 128),
                                              bass.ts(ic, CH)])
                                nc.vector.tensor_add(s2[:, t, :],
                                                     s2[:, t, :], mt[:])
                        es2 = esp.tile([128, 2, CH], BF16, tag="es",
                                       name="es2")
                        nc.scalar.activation(es2[:], s2[:], EXPF)
                        pend.append((h, jp, q0s, es2, sum_ps, o_ps))
                        if len(pend) > 1:
                            drain_one()
                while pend:
                    drain_one()
        # ---------- o_proj  po = wo^T @ attnT (WO aliased in WQ) ----------
        with tc.tile_pool(name="o_out", bufs=8) as outp, \
             tc.tile_pool(name="o_ps", bufs=8, space="PSUM") as psp:
            NB = 4           # n-tiles per block; kl-outer within a block
            for ic in range(NCH):
                for nb in range(H // 128 // NB):
                    pps = [psp.tile([128, CH], F32, tag="ps", name="pps")
                           for _ in range(NB)]
                    for kl in range(HPC):
                        for i in range(NB):
                            n = nb * NB + i
                            wo_ap = WQ[:, kl * 8 + n // 4,
                                       (n % 4) * 128:(n % 4) * 128 + 128]
                            mm(pps[i][:], wo_ap,
                               AT[ic][:, kl, :],
                               kl == 0, kl == HPC - 1)
                    for i in range(NB):
                        n = nb * NB + i
                        ot = outp.tile([128, CH], BF16, tag="ot", name="ot")
                        nc.scalar.copy(out=ot[:], in_=pps[i][:])
                        nc.gpsimd.dma_start(
                            out=po[bass.ts(n, 128), bass.ts(ic, CH)],
                            in_=ot[:])
        es_res.close()
        at_pool_cm.__exit__(None, None, None)
    nc.compile()
    return nc


_CACHE = {}


def _get_nc(causal):
    if causal not in _CACHE:
        _CACHE[causal] = _build(causal)
    return _CACHE[causal]


def kernel(hidden_states, attention_mask, position_ids, Wq, Wk, Wv, Wo):
    global last_exec_time_ns
    B, S_, H_ = hidden_states.shape
    assert (B, S_, H_) == (1, S, H)
    hs = np.asarray(hidden_states, dtype=np.float32)
    mask = np.asarray(attention_mask, dtype=np.float32)[0, 0]
    pos = np.asarray(position_ids)[0].astype(np.float64)

    # causal-mask fast path check
    iu = np.triu_indices(S, k=1)
    il = np.tril_indices(S, k=0)
    causal = bool(np.all(mask[il] == 0.0) and np.all(mask[iu] <= -1e30))

    hT = np.asarray(hs[0]).T               # [H, S]
    scale = 1.0 / np.sqrt(D)

    inv_freq = 1.0 / (ROPE_BASE ** (np.arange(0, D, 2, dtype=np.float64) / D))
    ang = pos[None, :] * np.concatenate([inv_freq, inv_freq])[:, None]  # [D,S]
    cosb = _b(np.cos(ang))
    sgn = np.ones((D, 1)); sgn[: D // 2] = -1.0
    sinb = _b(np.sin(ang) * sgn)

    htb = _pmajor(_b(hT), KT_TILES)
    wq_s = _b(np.asarray(Wq, np.float64) * scale)
    wk_b = _b(Wk)
    wv_b = _b(Wv)
    wo_b = _b(Wo)

    if causal:
        # band mask tile [128, 896]: mb[r, y] = NEG iff r > y - 384
        rr = np.arange(128)[:, None]
        yy = np.arange(896)[None, :]
        mband = np.ascontiguousarray(
            np.where(rr > yy - 384, NEG, 0.0).astype(BFNP))
    else:
        maskT = _r(mask.T)

    nc = _get_nc(causal)
    in_maps = []
    for c in range(N_CORES):
        sl = slice(c * HC, (c + 1) * HC)
        m = {
            "htb": htb,
            "wqb": _pmajor(wq_s[:, sl], KT_TILES),
            "wkb": _pmajor(wk_b[:, sl], KT_TILES),
            "wvb": np.ascontiguousarray(
                wv_b[:, sl].reshape(8, 4, 128, HC).transpose(0, 2, 1, 3)
                .reshape(8, 128, 4 * HC)),
            "wob": _pmajor(wo_b[sl, :], HPC),
            "cosb": cosb,
            "sinb": sinb,
        }
        if causal:
            m["mband"] = mband
        else:
            m["maskT"] = maskT
        in_maps.append(m)

    trace = bool(int(os.environ.get("BASS_KERNEL_TRACE", "0")))
    kw = {}
    if trace:
        kw["trace"] = True
        kw["tmpdir"] = os.environ.get("BASS_KERNEL_TRACE_DIR") or None
    res = run_bass_kernel_spmd(nc, in_maps, list(range(N_CORES)), **kw)
    last_exec_time_ns = res.exec_time_ns

    acc = np.zeros((H, S), dtype=np.float32)
    for c in range(N_CORES):
        acc += res.results[c]["po"].astype(np.float32)
    out = acc.T.reshape(1, S, H)
    return out
